# revision 4
# baseline (speedup 1.0000x reference)
"""Trainium2 Bass kernel for the sparse-attention module.

Reference computation (per batch element b):
    q = wq @ x + bq ; k = wk @ x + bk ; v = wv @ x + bv        # [S, N]
    att[i, j] = softmax_j( sum_s k[s, i] * q[s, j] )           # [N, N]
    v2 = v @ att                                               # [S, N]
    out = coef * (wa @ v2 + ba) + x                            # [C, N]
returns (out, att).

Sharding: pure data parallel over batch — B=8 batch elements, one per
NeuronCore. Params are tiny and replicated (pre-transposed on host into
PE-friendly layouts).

Per-core dataflow (C=512, N=4096, S=64):
  phase A: DMA x in, project q/k/v on PE (f32r matmuls -> bf16 q/k/v),
           PE-transpose v -> vT (bf16)
  phase B: per 128-row tile of att: PE k^T q (bf16) -> PSUM f32, ACT
           exp(+row sums) -> bf16 SBUF, DVE 1/sum scale (bf16 4x mode),
           DMA att rows out as bf16 (host widens to f32), PE v2
           accumulation (bf16, col-tiled into both PSUM partition halves)
  phase C: PE wa @ v2 (f32r), DVE epilogue coef*sa + x + coef*ba, DMA out
"""

import numpy as np

B, C, H, W = 8, 512, 64, 64
N = H * W          # 4096 tokens
S = C // 8         # 64   small channels
CK = C // 128      # 4    c-chunks (partition blocks of x / out)
NT = N // 128      # 32   n-tiles (att row blocks)
MC = N // 512      # 8    m-chunks (512-wide matmul free dim)

_CACHED = None


def _build():
    import concourse.tile as tile
    from concourse import bacc, mybir

    F32 = mybir.dt.float32
    F32R = mybir.dt.float32r
    BF16 = mybir.dt.bfloat16
    ADD = mybir.AluOpType.add
    MULT = mybir.AluOpType.mult
    EXP = mybir.ActivationFunctionType.Exp
    AX = mybir.AxisListType.X

    nc = bacc.Bacc("TRN2", target_bir_lowering=False, debug=False)

    x_d = nc.dram_tensor("x", [C, N], F32R, kind="ExternalInput")
    wqT_d = nc.dram_tensor("wqT", [C, S], F32R, kind="ExternalInput")
    wkT_d = nc.dram_tensor("wkT", [C, S], F32R, kind="ExternalInput")
    wvT_d = nc.dram_tensor("wvT", [C, S], F32R, kind="ExternalInput")
    bq_d = nc.dram_tensor("bq", [S, 1], F32, kind="ExternalInput")
    bk_d = nc.dram_tensor("bk", [S, 1], F32, kind="ExternalInput")
    bv_d = nc.dram_tensor("bv", [S, 1], F32, kind="ExternalInput")
    waT_d = nc.dram_tensor("waT", [S, C], F32R, kind="ExternalInput")
    ba4_d = nc.dram_tensor("ba4", [128, CK], F32, kind="ExternalInput")
    coef_d = nc.dram_tensor("coefb", [128, 1], F32, kind="ExternalInput")
    id_d = nc.dram_tensor("ident", [S, S], BF16, kind="ExternalInput")

    out_d = nc.dram_tensor("out", [C, N], F32, kind="ExternalOutput")
    att_d = nc.dram_tensor("att", [N, N], BF16, kind="ExternalOutput")

    with tile.TileContext(nc) as tc:
        with (
            tc.tile_pool(name="persist", bufs=1) as pp,
            tc.tile_pool(name="attp", bufs=3) as attp,
            tc.tile_pool(name="outp", bufs=2) as outp,
            tc.tile_pool(name="stats", bufs=4) as statp,
        ):
            x_sb = pp.tile([128, CK, N], F32R)
            q_sb = pp.tile([S, N], BF16)
            k_sb = pp.tile([S, N], BF16)
            v_sb = pp.tile([S, N], BF16)
            vT_sb = pp.tile([128, NT, S], BF16)
            v2_sb = pp.tile([S, N], F32R)
            v2h_sb = pp.tile([128, 4 * 512], F32R)
            wqT_sb = pp.tile([128, CK, S], F32R)
            wkT_sb = pp.tile([128, CK, S], F32R)
            wvT_sb = pp.tile([128, CK, S], F32R)
            waT_sb = pp.tile([S, C], F32R)
            bq_sb = pp.tile([S, 1], F32)
            bk_sb = pp.tile([S, 1], F32)
            bv_sb = pp.tile([S, 1], F32)
            ba4_sb = pp.tile([128, CK], F32)
            coef_sb = pp.tile([128, 1], F32)
            bacoef = pp.tile([128, CK], F32)
            ident_sb = pp.tile([S, S], BF16)

            # stream x in column chunks so the first q/k/v matmuls can start
            # as soon as the first 512 tokens of all four c-chunks are in
            x_re = x_d.ap().rearrange("(kk p) n -> p kk n", p=128)
            for j in range(MC):
                for kk in range(CK):
                    nc.sync.dma_start(
                        x_sb[:, kk, 512 * j : 512 * (j + 1)],
                        x_re[:, kk, 512 * j : 512 * (j + 1)],
                    )
            for w_sb, w_d in ((wqT_sb, wqT_d), (wkT_sb, wkT_d), (wvT_sb, wvT_d)):
                nc.sync.dma_start(
                    w_sb[:], w_d.ap().rearrange("(kk p) s -> p kk s", p=128)
                )
            nc.sync.dma_start(bq_sb[:], bq_d.ap())
            nc.sync.dma_start(bk_sb[:], bk_d.ap())
            nc.sync.dma_start(bv_sb[:], bv_d.ap())
            nc.sync.dma_start(waT_sb[:], waT_d.ap())
            nc.sync.dma_start(ba4_sb[:], ba4_d.ap())
            nc.sync.dma_start(coef_sb[:], coef_d.ap())
            nc.sync.dma_start(ident_sb[:], id_d.ap())

            nc.vector.tensor_scalar_mul(bacoef[:], ba4_sb[:], coef_sb[:])

            # ---------------- phase A: q/k/v projections + v transpose
            with tc.tile_pool(name="psA", bufs=4, space="PSUM") as psA:
                for w_sb, b_sb, dst in (
                    (wqT_sb, bq_sb, q_sb),
                    (wkT_sb, bk_sb, k_sb),
                    (wvT_sb, bv_sb, v_sb),
                ):
                    for j in range(MC):
                        ps = psA.tile([S, 512], F32, tag="qk")
                        for kk in range(CK):
                            nc.tensor.matmul(
                                ps[:],
                                w_sb[:, kk, :],
                                x_sb[:, kk, 512 * j : 512 * (j + 1)],
                                start=(kk == 0),
                                stop=(kk == CK - 1),
                            )
                        nc.vector.tensor_scalar_add(
                            dst[:, 512 * j : 512 * (j + 1)], ps[:], b_sb[:]
                        )
                for i in range(NT):
                    pst = psA.tile([128, S], BF16, tag="tp")
                    nc.tensor.transpose(
                        pst[:], v_sb[:, 128 * i : 128 * (i + 1)], ident_sb[:]
                    )
                    nc.vector.tensor_copy(vT_sb[:, i, :], pst[:])

            # ---------------- phase B: attention rows + v2 accumulation
            with (
                tc.tile_pool(name="psB", bufs=1, space="PSUM") as psB,
                tc.tile_pool(name="psV", bufs=1, space="PSUM") as psV,
            ):
                v2ps = psV.tile([128, 4 * 512], F32)

                def emit_v2(i, asb_i, jlist):
                    # interleave the two psum col-groups so the pairs run
                    # concurrently on the PE quadrants
                    for j in jlist:
                        rb = 64 * (j // 4)
                        nc.tensor.matmul(
                            v2ps[rb : rb + 64, 512 * (j % 4) : 512 * (j % 4) + 512],
                            vT_sb[:, i, :],
                            asb_i[:, 512 * j : 512 * (j + 1)],
                            start=(i == 0),
                            stop=(i == NT - 1),
                            tile_position=(0, rb),
                            skip_group_check=True,
                        )

                prev = None  # (i, asb) whose v2 matmuls are still pending
                for i in range(NT):
                    asb = attp.tile([128, N], BF16)
                    st = statp.tile([128, 4], F32)
                    for h in range(2):
                        aps = psB.tile([128, 2048], F32)
                        for j in range(4):
                            m0 = 2048 * h + 512 * j
                            nc.tensor.matmul(
                                aps[:, 512 * j : 512 * (j + 1)],
                                k_sb[:, 128 * i : 128 * (i + 1)],
                                q_sb[:, m0 : m0 + 512],
                                start=True,
                                stop=True,
                            )
                        nc.scalar.activation(
                            asb[:, 2048 * h : 2048 * (h + 1)],
                            aps[:],
                            EXP,
                            accum_out=st[:, h : h + 1],
                        )
                        if h == 0 and prev is not None:
                            # fill the exp(h0) wait with the previous tile's
                            # v2 accumulation — keeps the PE dense (HAM warm)
                            emit_v2(prev[0], prev[1], [0, 4, 1, 5])
                    nc.vector.reduce_sum(st[:, 2:3], st[:, 0:2], axis=AX)
                    nc.vector.reciprocal(st[:, 3:4], st[:, 2:3])
                    nc.vector.tensor_scalar_mul(asb[:], asb[:], st[:, 3:4])
                    nc.sync.dma_start(att_d.ap()[128 * i : 128 * (i + 1), :], asb[:])
                    if prev is not None:
                        emit_v2(prev[0], prev[1], [2, 6, 3, 7])
                    prev = (i, asb)
                emit_v2(prev[0], prev[1], [0, 4, 1, 5, 2, 6, 3, 7])
                # v2ps rows 0-63 hold m-chunks 0-3; rows 64-127 hold 4-7.
                # Everything must land on partitions 0-63 for the sa matmuls:
                # lower half straight via DVE, upper half DVE->SBUF then a
                # cross-partition SBUF->SBUF DMA.
                nc.vector.tensor_copy(v2_sb[:, 0 : N // 2], v2ps[0:64, :])
                nc.vector.tensor_copy(v2h_sb[64:128, :], v2ps[64:128, :])
                nc.sync.dma_start(v2_sb[:, N // 2 : N], v2h_sb[64:128, :])

            # ---------------- phase C: sa = wa @ v2, epilogue, out
            with tc.tile_pool(name="psD", bufs=2, space="PSUM") as psD:
                for kk in range(CK):
                    for h in range(2):
                        sps = psD.tile([128, 2048], F32)
                        for j in range(4):
                            jj = 4 * h + j
                            nc.tensor.matmul(
                                sps[:, 512 * j : 512 * (j + 1)],
                                waT_sb[:, 128 * kk : 128 * (kk + 1)],
                                v2_sb[:, 512 * jj : 512 * (jj + 1)],
                                start=True,
                                stop=True,
                            )
                        osb = outp.tile([128, 2048], F32)
                        nc.vector.scalar_tensor_tensor(
                            osb[:],
                            sps[:],
                            coef_sb[:],
                            x_sb[:, kk, 2048 * h : 2048 * (h + 1)].bitcast(F32),
                            op0=MULT,
                            op1=ADD,
                        )
                        nc.vector.tensor_scalar_add(
                            osb[:], osb[:], bacoef[:, kk : kk + 1]
                        )
                        nc.sync.dma_start(
                            out_d.ap()[
                                128 * kk : 128 * (kk + 1), 2048 * h : 2048 * (h + 1)
                            ],
                            osb[:],
                        )

    nc.compile()
    return nc


def _get_nc():
    global _CACHED
    if _CACHED is None:
        _CACHED = _build()
    return _CACHED


def make_in_maps(x, wq, bq, wk, bk, wv, bv, wa, ba, coef):
    import ml_dtypes

    x = np.asarray(x, dtype=np.float32)
    xf = np.ascontiguousarray(x.reshape(B, C, N))
    shared = {
        "wqT": np.ascontiguousarray(np.asarray(wq, np.float32).T),
        "wkT": np.ascontiguousarray(np.asarray(wk, np.float32).T),
        "wvT": np.ascontiguousarray(np.asarray(wv, np.float32).T),
        "bq": np.ascontiguousarray(np.asarray(bq, np.float32).reshape(S, 1)),
        "bk": np.ascontiguousarray(np.asarray(bk, np.float32).reshape(S, 1)),
        "bv": np.ascontiguousarray(np.asarray(bv, np.float32).reshape(S, 1)),
        "waT": np.ascontiguousarray(np.asarray(wa, np.float32).T),
        "ba4": np.ascontiguousarray(np.asarray(ba, np.float32).reshape(CK, 128).T),
        "coefb": np.full((128, 1), np.float32(np.asarray(coef).reshape(-1)[0])),
        "ident": np.eye(S, dtype=ml_dtypes.bfloat16),
    }
    return [dict(shared, x=np.ascontiguousarray(xf[b])) for b in range(B)]


def kernel(x, wq, bq, wk, bk, wv, bv, wa, ba, coef, **_unused):
    from concourse.bass_utils import run_bass_kernel_spmd

    nc = _get_nc()
    in_maps = make_in_maps(x, wq, bq, wk, bk, wv, bv, wa, ba, coef)
    res = run_bass_kernel_spmd(nc, in_maps, core_ids=list(range(B)))

    out = np.stack([res.results[b]["out"].reshape(C, H, W) for b in range(B)])
    att = np.stack(
        [res.results[b]["att"].astype(np.float32) for b in range(B)]
    )
    return out, att


# revision 6
# speedup vs baseline: 1.4118x; 1.4118x over previous
"""Trainium2 Bass kernel for the sparse-attention module.

Reference computation (per batch element b):
    q = wq @ x + bq ; k = wk @ x + bk ; v = wv @ x + bv        # [S, N]
    att[i, j] = softmax_j( sum_s k[s, i] * q[s, j] )           # [N, N]
    v2 = v @ att                                               # [S, N]
    out = coef * (wa @ v2 + ba) + x                            # [C, N]
returns (out, att).

Sharding: pure data parallel over batch — B=8 batch elements, one per
NeuronCore. Params are tiny and replicated (pre-transposed on host into
PE-friendly layouts).

Per-core dataflow (C=512, N=4096, S=64):
  phase A: DMA x in, project q/k/v on PE (f32r matmuls -> bf16 q/k/v),
           PE-transpose v -> vT (bf16)
  phase B: per 128-row tile of att: PE k^T q (bf16) -> PSUM f32, ACT
           exp(+row sums) -> bf16 SBUF, DVE 1/sum scale (bf16 4x mode),
           DMA att rows out as bf16 (host widens to f32), PE v2
           accumulation (bf16, col-tiled into both PSUM partition halves)
  phase C: PE wa @ v2 (f32r), DVE epilogue coef*sa + x + coef*ba, DMA out
"""

import numpy as np

B, C, H, W = 8, 512, 64, 64
N = H * W          # 4096 tokens
S = C // 8         # 64   small channels
CK = C // 128      # 4    c-chunks (partition blocks of x / out)
NT = N // 128      # 32   n-tiles (att row blocks)
MC = N // 512      # 8    m-chunks (512-wide matmul free dim)

_CACHED = None


def _build():
    import concourse.tile as tile
    from concourse import bacc, mybir

    F32 = mybir.dt.float32
    F32R = mybir.dt.float32r
    BF16 = mybir.dt.bfloat16
    ADD = mybir.AluOpType.add
    MULT = mybir.AluOpType.mult
    EXP = mybir.ActivationFunctionType.Exp
    AX = mybir.AxisListType.X

    nc = bacc.Bacc("TRN2", target_bir_lowering=False, debug=False)

    x_d = nc.dram_tensor("x", [C, N], F32R, kind="ExternalInput")
    wqT_d = nc.dram_tensor("wqT", [C, S], F32R, kind="ExternalInput")
    wkT_d = nc.dram_tensor("wkT", [C, S], F32R, kind="ExternalInput")
    wvT_d = nc.dram_tensor("wvT", [C, S], F32R, kind="ExternalInput")
    bq_d = nc.dram_tensor("bq", [S, 1], F32, kind="ExternalInput")
    bk_d = nc.dram_tensor("bk", [S, 1], F32, kind="ExternalInput")
    bv_d = nc.dram_tensor("bv", [S, 1], F32, kind="ExternalInput")
    waT_d = nc.dram_tensor("waT", [S, C], F32R, kind="ExternalInput")
    ba4_d = nc.dram_tensor("ba4", [128, CK], F32, kind="ExternalInput")
    coef_d = nc.dram_tensor("coefb", [128, 1], F32, kind="ExternalInput")
    id_d = nc.dram_tensor("ident", [S, S], BF16, kind="ExternalInput")

    out_d = nc.dram_tensor("out", [C, N], F32, kind="ExternalOutput")
    att_d = nc.dram_tensor("att", [N, N], BF16, kind="ExternalOutput")

    with tile.TileContext(nc) as tc:
        with (
            tc.tile_pool(name="persist", bufs=1) as pp,
            tc.tile_pool(name="attp", bufs=3) as attp,
            tc.tile_pool(name="outp", bufs=2) as outp,
            tc.tile_pool(name="stats", bufs=4) as statp,
        ):
            x_sb = pp.tile([128, CK, N], F32R)
            q_sb = pp.tile([S, N], BF16)
            k_sb = pp.tile([S, N], BF16)
            v_sb = pp.tile([S, N], BF16)
            vT_sb = pp.tile([128, NT, S], BF16)
            v2_sb = pp.tile([S, N], F32R)
            v2h_sb = pp.tile([128, 4 * 512], F32R)
            wqT_sb = pp.tile([128, CK, S], F32R)
            wkT_sb = pp.tile([128, CK, S], F32R)
            wvT_sb = pp.tile([128, CK, S], F32R)
            waT_sb = pp.tile([S, C], F32R)
            bq_sb = pp.tile([S, 1], F32)
            bk_sb = pp.tile([S, 1], F32)
            bv_sb = pp.tile([S, 1], F32)
            ba4_sb = pp.tile([128, CK], F32)
            coef_sb = pp.tile([128, 1], F32)
            bacoef = pp.tile([128, CK], F32)
            ident_sb = pp.tile([S, S], BF16)

            # stream x in column chunks so the first q/k/v matmuls can start
            # as soon as the first 512 tokens of all four c-chunks are in
            x_re = x_d.ap().rearrange("(kk p) n -> p kk n", p=128)
            for j in range(MC):
                for kk in range(CK):
                    nc.sync.dma_start(
                        x_sb[:, kk, 512 * j : 512 * (j + 1)],
                        x_re[:, kk, 512 * j : 512 * (j + 1)],
                    )
            for w_sb, w_d in ((wqT_sb, wqT_d), (wkT_sb, wkT_d), (wvT_sb, wvT_d)):
                nc.sync.dma_start(
                    w_sb[:], w_d.ap().rearrange("(kk p) s -> p kk s", p=128)
                )
            nc.sync.dma_start(bq_sb[:], bq_d.ap())
            nc.sync.dma_start(bk_sb[:], bk_d.ap())
            nc.sync.dma_start(bv_sb[:], bv_d.ap())
            nc.sync.dma_start(waT_sb[:], waT_d.ap())
            nc.sync.dma_start(ba4_sb[:], ba4_d.ap())
            nc.sync.dma_start(coef_sb[:], coef_d.ap())
            nc.sync.dma_start(ident_sb[:], id_d.ap())

            nc.vector.tensor_scalar_mul(bacoef[:], ba4_sb[:], coef_sb[:])

            # ---------------- phase A: q/k/v projections + v transpose
            with tc.tile_pool(name="psA", bufs=4, space="PSUM") as psA:
                for w_sb, b_sb, dst in (
                    (wqT_sb, bq_sb, q_sb),
                    (wkT_sb, bk_sb, k_sb),
                    (wvT_sb, bv_sb, v_sb),
                ):
                    for j in range(MC):
                        ps = psA.tile([S, 512], F32, tag="qk")
                        for kk in range(CK):
                            nc.tensor.matmul(
                                ps[:],
                                w_sb[:, kk, :],
                                x_sb[:, kk, 512 * j : 512 * (j + 1)],
                                start=(kk == 0),
                                stop=(kk == CK - 1),
                            )
                        nc.vector.tensor_scalar_add(
                            dst[:, 512 * j : 512 * (j + 1)], ps[:], b_sb[:]
                        )
                for i in range(NT):
                    pst = psA.tile([128, S], BF16, tag="tp")
                    nc.tensor.transpose(
                        pst[:], v_sb[:, 128 * i : 128 * (i + 1)], ident_sb[:]
                    )
                    nc.vector.tensor_copy(vT_sb[:, i, :], pst[:])

            # ---------------- phase B: attention rows + v2 accumulation
            with (
                tc.tile_pool(name="psB", bufs=2, space="PSUM") as psB,
                tc.tile_pool(name="psV", bufs=1, space="PSUM") as psV,
            ):
                v2ps = psV.tile([128, 4 * 512], F32)

                def emit_v2(i, asb_i, jlist):
                    # interleave the two psum col-groups so the pairs run
                    # concurrently on the PE quadrants
                    for j in jlist:
                        rb = 64 * (j // 4)
                        nc.tensor.matmul(
                            v2ps[rb : rb + 64, 512 * (j % 4) : 512 * (j % 4) + 512],
                            vT_sb[:, i, :],
                            asb_i[:, 512 * j : 512 * (j + 1)],
                            start=(i == 0),
                            stop=(i == NT - 1),
                            tile_position=(0, rb),
                            skip_group_check=True,
                        )

                prev = None  # (i, asb) whose v2 matmuls are still pending
                for i in range(NT):
                    asb = attp.tile([128, N], BF16)
                    st = statp.tile([128, 8], F32)
                    # quarter-granularity att: [128,1024] psum double-buffered
                    # so the exp of quarter q overlaps the matmuls of q+1
                    for qq in range(4):
                        aps = psB.tile([128, 1024], F32)
                        for j in range(2):
                            m0 = 1024 * qq + 512 * j
                            nc.tensor.matmul(
                                aps[:, 512 * j : 512 * (j + 1)],
                                k_sb[:, 128 * i : 128 * (i + 1)],
                                q_sb[:, m0 : m0 + 512],
                                start=True,
                                stop=True,
                            )
                        nc.scalar.activation(
                            asb[:, 1024 * qq : 1024 * (qq + 1)],
                            aps[:],
                            EXP,
                            accum_out=st[:, qq : qq + 1],
                        )
                        if qq == 1 and prev is not None:
                            emit_v2(prev[0], prev[1], [0, 4, 1, 5])
                        if qq == 3 and prev is not None:
                            emit_v2(prev[0], prev[1], [2, 6])
                    nc.vector.reduce_sum(st[:, 4:5], st[:, 0:4], axis=AX)
                    nc.vector.reciprocal(st[:, 5:6], st[:, 4:5])
                    nc.vector.tensor_scalar_mul(asb[:], asb[:], st[:, 5:6])
                    nc.sync.dma_start(att_d.ap()[128 * i : 128 * (i + 1), :], asb[:])
                    if prev is not None:
                        emit_v2(prev[0], prev[1], [3, 7])
                    prev = (i, asb)
                emit_v2(prev[0], prev[1], [0, 4, 1, 5, 2, 6, 3, 7])
                # v2ps rows 0-63 hold m-chunks 0-3; rows 64-127 hold 4-7.
                # Everything must land on partitions 0-63 for the sa matmuls:
                # lower half straight via DVE, upper half DVE->SBUF then a
                # cross-partition SBUF->SBUF DMA.
                nc.vector.tensor_copy(v2_sb[:, 0 : N // 2], v2ps[0:64, :])
                nc.vector.tensor_copy(v2h_sb[64:128, :], v2ps[64:128, :])
                nc.sync.dma_start(v2_sb[:, N // 2 : N], v2h_sb[64:128, :])

            # ---------------- phase C: sa = wa @ v2, epilogue, out
            with tc.tile_pool(name="psD", bufs=2, space="PSUM") as psD:
                for kk in range(CK):
                    for h in range(2):
                        sps = psD.tile([128, 2048], F32)
                        for j in range(4):
                            jj = 4 * h + j
                            nc.tensor.matmul(
                                sps[:, 512 * j : 512 * (j + 1)],
                                waT_sb[:, 128 * kk : 128 * (kk + 1)],
                                v2_sb[:, 512 * jj : 512 * (jj + 1)],
                                start=True,
                                stop=True,
                            )
                        osb = outp.tile([128, 2048], F32)
                        nc.vector.scalar_tensor_tensor(
                            osb[:],
                            sps[:],
                            coef_sb[:],
                            x_sb[:, kk, 2048 * h : 2048 * (h + 1)].bitcast(F32),
                            op0=MULT,
                            op1=ADD,
                        )
                        nc.vector.tensor_scalar_add(
                            osb[:], osb[:], bacoef[:, kk : kk + 1]
                        )
                        nc.sync.dma_start(
                            out_d.ap()[
                                128 * kk : 128 * (kk + 1), 2048 * h : 2048 * (h + 1)
                            ],
                            osb[:],
                        )

    nc.compile()
    return nc


def _get_nc():
    global _CACHED
    if _CACHED is None:
        _CACHED = _build()
    return _CACHED


def make_in_maps(x, wq, bq, wk, bk, wv, bv, wa, ba, coef):
    import ml_dtypes

    x = np.asarray(x, dtype=np.float32)
    xf = np.ascontiguousarray(x.reshape(B, C, N))
    shared = {
        "wqT": np.ascontiguousarray(np.asarray(wq, np.float32).T),
        "wkT": np.ascontiguousarray(np.asarray(wk, np.float32).T),
        "wvT": np.ascontiguousarray(np.asarray(wv, np.float32).T),
        "bq": np.ascontiguousarray(np.asarray(bq, np.float32).reshape(S, 1)),
        "bk": np.ascontiguousarray(np.asarray(bk, np.float32).reshape(S, 1)),
        "bv": np.ascontiguousarray(np.asarray(bv, np.float32).reshape(S, 1)),
        "waT": np.ascontiguousarray(np.asarray(wa, np.float32).T),
        "ba4": np.ascontiguousarray(np.asarray(ba, np.float32).reshape(CK, 128).T),
        "coefb": np.full((128, 1), np.float32(np.asarray(coef).reshape(-1)[0])),
        "ident": np.eye(S, dtype=ml_dtypes.bfloat16),
    }
    return [dict(shared, x=np.ascontiguousarray(xf[b])) for b in range(B)]


def kernel(x, wq, bq, wk, bk, wv, bv, wa, ba, coef, **_unused):
    from concourse.bass_utils import run_bass_kernel_spmd

    nc = _get_nc()
    in_maps = make_in_maps(x, wq, bq, wk, bk, wv, bv, wa, ba, coef)
    res = run_bass_kernel_spmd(nc, in_maps, core_ids=list(range(B)))

    out = np.stack([res.results[b]["out"].reshape(C, H, W) for b in range(B)])
    att = np.stack(
        [res.results[b]["att"].astype(np.float32) for b in range(B)]
    )
    return out, att
